# revision 33
# baseline (speedup 1.0000x reference)
"""Trainium2 Bass kernel for a 2-layer LSTM encoder + autoregressive decoder.

Problem: batch 8192, 48 encode steps, 12 decode steps with BG-channel
feedback, hidden 128, input dim 8, fc head to 1 output.

Strategy: pure data parallelism over 8 NeuronCores (1024 batch rows each).
Hidden units sit on SBUF partitions, batch (1024) on the free dim in two
512-wide matmul streams (ISA moving-dim cap is 512):

  gates[512, B] = W_ih^T.T @ x[K, B] + W_hh^T.T @ h[128, B]   (PSUM accum)

The kernel is ScalarE(ACT)-bound: the 5 LUT activations per layer-step
over [128, 1024] are ~5us of mandatory ACT-engine streaming (1 elem/
cycle/lane, LUT exists only on ScalarE).  Everything serves keeping
ScalarE fed:

- x is zero-padded from 9 to 128 contraction rows (KX): K=9 matmuls run
  the PE array at 7% activity, which parks the HAM clock-gate at half
  rate for the whole loop; uniform K=128 work holds the PE at 2.4 GHz
  (216ns vs 427+ns per N=512 matmul).
- Per-gate 2-bank PSUM tiles, 2 tags x 2 bufs = all 8 banks, rolling
  per-gate across layer-steps.  ScalarE order [g, i, f, o, c] lets the
  DVE c-chain (t1, u, add) drain under sig_f/sig_o.
- Layer 1 emits all always-ready h1-part matmuls before the fresh-h0
  x-parts (the in-order PE queue would head-of-line-block otherwise).
- The c-update/tanh(c)/h tail is stream-split on the recurrence-critical
  paths (layer 0 always; layer 1 in decode, where h1 feeds the BG
  feedback) so the next quantum's matmuls start half an op earlier.
- Decode BG feedback is folded through the BG column as the rank-1
  weight wbgfc = outer(W_fc, W_bg) applied to last-step h1; b_fc * W_bg
  is folded into w9dec's bias row.  Feedback matmuls ride last in each
  gate's accumulation group (their rhs lands latest).
- Layer-0 biases ride x's 9th channel; layer-1 biases use the ACT
  per-partition bias operand.  Zero h/c at t=0 skip their matmuls.
- The fc head runs off-loop over all 12 decode steps with diagonal-
  extended weights [128, 12] accumulating into one [12, 1024] PSUM tile
  -> a single ACT copy (+b_fc) -> DMA out.
- Startup: one DMA queue drains serially, so step-0's critical tensors
  (w9t0, whht0, x step-0 chunk) are issued first and decode-only
  tensors last; first activation fires at ~9us instead of ~15us.

Weight columns are reordered host-side to [g, i, f, o].
Measured: 753us HW exec (baseline 814us), rel err 1.24e-3.  ScalarE is
86% busy on ~640us of LUT streaming; the residual idle is pure recurrence
dependency latency (the ~1.1us h-turnaround per step has no fillable work:
ScalarE is packed tail-to-head up to each gap, so added ops anywhere only
relocate it -- measured twice).
"""

import sys

sys.path.insert(0, "/opt/trn_rl_repo")

import numpy as np

import concourse.bacc as bacc
import concourse.tile as tile
from concourse import mybir
from concourse import bass_utils
from concourse.bass import ts

F16 = np.float16

B_TOTAL = 8192
T = 60
T_ENC = 48
T_DEC = 12
DIN = 8
H = 128
NG = 4 * H
N_CORES = 8
BSH = B_TOTAL // N_CORES  # 1024 batch rows per core
NS = 2  # batch streams
SB = BSH // NS  # 512
XT_STEPS = 8  # timesteps per streamed x tile
KX = 128  # x rows zero-padded 9 -> 128: K=9 matmuls run the PE array at 7%
          # activity, which keeps the HAM clock-gate at half rate; uniform
          # K=128 work holds the PE at 2.4 GHz

# gate slots after host-side column reorder: g, i, f, o
SG, SI, SF, SO = 0, 1, 2, 3

_CACHE: dict = {}


def _build(bfc: float):
    f32 = mybir.dt.float32
    f16 = mybir.dt.float16

    nc = bacc.Bacc("TRN2", debug=False, num_devices=N_CORES)

    x_d = nc.dram_tensor("x", [KX, T, NS, SB], f16, kind="ExternalInput")
    w9t0_d = nc.dram_tensor("w9t0", [KX, NG], f16, kind="ExternalInput")
    w9dec_d = nc.dram_tensor("w9dec", [KX, NG], f16, kind="ExternalInput")
    whht0_d = nc.dram_tensor("whht0", [H, NG], f16, kind="ExternalInput")
    wiht1_d = nc.dram_tensor("wiht1", [H, NG], f16, kind="ExternalInput")
    whht1_d = nc.dram_tensor("whht1", [H, NG], f16, kind="ExternalInput")
    wbg0t_d = nc.dram_tensor("wbg0t", [H, NG], f16, kind="ExternalInput")
    wbgfc_d = nc.dram_tensor("wbgfc", [H, NG], f16, kind="ExternalInput")
    wfcd_d = nc.dram_tensor("wfcd", [H, T_DEC, T_DEC], f16, kind="ExternalInput")
    b1_d = nc.dram_tensor("b1", [H, 4], f32, kind="ExternalInput")
    bg0_d = nc.dram_tensor("bg0", [H, NS, SB], f16, kind="ExternalInput")
    out_d = nc.dram_tensor("out", [T_DEC, NS, SB], f32, kind="ExternalOutput")

    SIG = mybir.ActivationFunctionType.Sigmoid
    TANH = mybir.ActivationFunctionType.Tanh
    COPY = mybir.ActivationFunctionType.Copy

    with tile.TileContext(nc) as tc:
        with (
            tc.tile_pool(name="wpool", bufs=1) as wpool,
            tc.tile_pool(name="xpool", bufs=3) as xpool,
            tc.tile_pool(name="state", bufs=1) as state,
            tc.tile_pool(name="gates", bufs=3) as gates,
            tc.tile_pool(name="psum", bufs=2, space="PSUM") as psum,
        ):
            w9t0 = wpool.tile([KX, NG], f16)
            w9dec = wpool.tile([KX, NG], f16)
            whht0 = wpool.tile([H, NG], f16)
            wiht1 = wpool.tile([H, NG], f16)
            whht1 = wpool.tile([H, NG], f16)
            wbg0t = wpool.tile([H, NG], f16)
            wbgfc = wpool.tile([H, NG], f16)
            wfcd = wpool.tile([H, T_DEC, T_DEC], f16)
            b1 = wpool.tile([H, 4], f32)
            # Startup: a dummy sigmoid first makes the one table set that
            # holds BOTH sigmoid and tanh resident in a single
            # ACT_TABLE_LOAD (tanh-first loads tanh's home set, then the
            # sigmoid set replaces it: two loads).  Junk matmuls on the
            # scratch tile warm the PE clock-gate to 2.4 GHz while the
            # first weight/x DMAs drain.
            scratch = wpool.tile([H, SB], f16)
            nc.vector.memset(scratch[:], 0.25)
            nc.scalar.activation(scratch[:, 1:2], scratch[:, 0:1], SIG)
            # DMA order matters: one queue drains these serially.  Step 0
            # skips its (zero) h-parts, so only w9t0 + the first x chunk
            # gate the first matmuls; whht0 is needed a full step later.
            nc.sync.dma_start(w9t0[:], w9t0_d.ap())

            # recurrent state; h double-buffered on step parity
            h = [[None, None], [None, None]]  # h[layer][parity]
            c = [None, None]
            for l in range(2):
                for p in range(2):
                    h[l][p] = state.tile([H, NS, SB], f16, name=f"h_{l}_{p}")
                    nc.vector.memset(h[l][p][:], 0.0)
                c[l] = state.tile([H, NS, SB], f16, name=f"c_{l}")
                nc.vector.memset(c[l][:], 0.0)
            # decode h1 states kept for the batched fc head
            h1dec = state.tile([H, T_DEC, NS, SB], f16, name="h1dec")
            bg = state.tile([H, NS, SB], f16, name="bg")

            def h1_read(t):
                """h1 produced at step t-1 (as read by step t)."""
                if t <= T_ENC:
                    return h[1][1 - (t % 2)]
                return h1dec[:, t - 1 - T_ENC]

            def quantum(t, layer, xt, tr):
                p = t % 2
                dec = t >= T_ENC

                # Per-gate 2-bank PSUM tiles [H, NS, SB]; 2 tags x 2 bufs
                # = all 8 banks, rolling per-gate across quanta.
                gps = {}
                for slot, tag in ((SG, "psA"), (SI, "psB"), (SF, "psA"), (SO, "psB")):
                    gps[slot] = psum.tile(
                        [H, NS, SB], f32, tag=tag, name=f"ps_{t}_{layer}_{slot}"
                    )

                # ---- matmuls (all K=128, N=512) ----
                if layer == 0:
                    w_x = w9dec if t > T_ENC else w9t0
                    h_prev = h[0][1 - p]
                    # gate-major, x parts then h parts per gate: everything
                    # is ready, so tanh(g) can fire after the first 4-6 MMs
                    for slot in (SG, SI, SF, SO):
                        for st in range(NS):
                            nc.tensor.matmul(
                                gps[slot][:, st, :], w_x[:, ts(slot, H)],
                                xt[:, tr, st, :], start=True, stop=False,
                            )
                        for st in range(NS):
                            nc.tensor.matmul(
                                gps[slot][:, st, :], whht0[:, ts(slot, H)],
                                h_prev[:, st, :], start=False,
                                stop=not (dec or t == T_ENC),
                            )
                        if t == T_ENC:
                            for st in range(NS):
                                nc.tensor.matmul(
                                    gps[slot][:, st, :], wbg0t[:, ts(slot, H)],
                                    bg[:, st, :], start=False, stop=True,
                                )
                        elif dec:
                            # feedback last: its rhs (h1[t-1]) lands latest
                            for st in range(NS):
                                nc.tensor.matmul(
                                    gps[slot][:, st, :], wbgfc[:, ts(slot, H)],
                                    h1_read(t)[:, st, :], start=False, stop=True,
                                )
                else:
                    # layer 1: all always-ready h1-parts first, then the
                    # fresh-h0 x-parts (the in-order PE queue would
                    # head-of-line-block on an early fresh-h0 matmul)
                    h_prev = h1_read(t)
                    h0_t = h[0][p]
                    if t > 0:  # h1 is zero at t=0
                        for slot in (SG, SI, SF, SO):
                            for st in range(NS):
                                nc.tensor.matmul(
                                    gps[slot][:, st, :], whht1[:, ts(slot, H)],
                                    h_prev[:, st, :], start=True, stop=False,
                                )
                    for slot in (SG, SI, SF, SO):
                        for st in range(NS):
                            nc.tensor.matmul(
                                gps[slot][:, st, :], wiht1[:, ts(slot, H)],
                                h0_t[:, st, :], start=t == 0, stop=True,
                            )

                # ---- activations + cell update (FD=1024 ops) ----
                if layer == 1 and dec:
                    h_new = h1dec[:, t - T_ENC]
                else:
                    h_new = h[layer][p]
                c_own = c[layer]

                g_sb = gates.tile([H, NS, SB], f16, tag="g")
                i_sb = gates.tile([H, NS, SB], f16, tag="i")
                f_sb = gates.tile([H, NS, SB], f16, tag="f")
                o_sb = gates.tile([H, NS, SB], f16, tag="o")
                th = gates.tile([H, NS, SB], f16, tag="th")
                t1 = gates.tile([H, NS, SB], f16, tag="t1")
                u = gates.tile([H, NS, SB], f16, tag="u")

                bias = {}
                if layer == 1:
                    bias = {s: {"bias": b1[:, s : s + 1]} for s in range(4)}
                # ScalarE order [g, i, f, o, c]: the DVE chain (t1, u, add)
                # runs under sig_f/sig_o, so tanh(c) is ready when reached.
                # The h-multiply splits per stream so stream A's x-matmuls
                # of the next quantum start half an h-op earlier.  (Splitting
                # tanh_g per stream was tried twice and lost: +2 ops/step of
                # overhead beats the sub-op latency gain.)
                nc.scalar.activation(g_sb[:], gps[SG][:], TANH, **bias.get(SG, {}))
                nc.scalar.activation(i_sb[:], gps[SI][:], SIG, **bias.get(SI, {}))
                nc.vector.tensor_mul(t1[:], i_sb[:], g_sb[:])
                nc.scalar.activation(f_sb[:], gps[SF][:], SIG, **bias.get(SF, {}))
                if layer == 0:
                    # layer 0 feeds the critical h0 turnaround: run the
                    # c-update, tanh(c) and h per stream so stream A's
                    # next-quantum matmuls start ~0.5us earlier
                    for st in range(NS):
                        nc.vector.tensor_mul(
                            u[:, st, :], f_sb[:, st, :], c_own[:, st, :]
                        )
                        nc.vector.tensor_add(
                            c_own[:, st, :], u[:, st, :], t1[:, st, :]
                        )
                    nc.scalar.activation(o_sb[:], gps[SO][:], SIG)
                    for st in range(NS):
                        nc.scalar.activation(th[:, st, :], c_own[:, st, :], TANH)
                        nc.vector.tensor_mul(
                            h_new[:, st, :], o_sb[:, st, :], th[:, st, :]
                        )
                elif dec:
                    # decode: h1 feeds next step's BG feedback, so stream-
                    # split the tail for latency like layer 0
                    for st in range(NS):
                        nc.vector.tensor_mul(
                            u[:, st, :], f_sb[:, st, :], c_own[:, st, :]
                        )
                        nc.vector.tensor_add(
                            c_own[:, st, :], u[:, st, :], t1[:, st, :]
                        )
                    nc.scalar.activation(
                        o_sb[:], gps[SO][:], SIG, **bias.get(SO, {})
                    )
                    for st in range(NS):
                        nc.scalar.activation(th[:, st, :], c_own[:, st, :], TANH)
                        nc.vector.tensor_mul(
                            h_new[:, st, :], o_sb[:, st, :], th[:, st, :]
                        )
                else:
                    # encode layer 1: h1 is only needed a full step later,
                    # so use the cheaper merged FD=1024 ops
                    nc.vector.tensor_mul(u[:], f_sb[:], c_own[:])
                    nc.vector.tensor_add(c_own[:], u[:], t1[:])
                    nc.scalar.activation(
                        o_sb[:], gps[SO][:], SIG, **bias.get(SO, {})
                    )
                    nc.scalar.activation(th[:], c_own[:], TANH)
                    nc.vector.tensor_mul(h_new[:], o_sb[:], th[:])

            # x tiles: 8-step tiles for encode, one 12-step tile for decode.
            # The first tile's DMA is split so step 0 starts after ~1/8 of
            # the transfer instead of the whole 2MB.
            xt = None
            t0 = 0
            x_tile_starts = list(range(0, T_ENC, XT_STEPS)) + [T_ENC]
            for t in range(T):
                if t in x_tile_starts:
                    t0 = t
                    nt = T_DEC if t == T_ENC else XT_STEPS
                    xt = xpool.tile([KX, T_DEC, NS, SB], f16)
                    if t == 0:
                        nc.sync.dma_start(xt[:, :1], x_d.ap()[:, 0:1])
                        nc.sync.dma_start(whht0[:], whht0_d.ap())
                        nc.sync.dma_start(wiht1[:], wiht1_d.ap())
                        nc.sync.dma_start(whht1[:], whht1_d.ap())
                        nc.sync.dma_start(b1[:], b1_d.ap())
                        nc.sync.dma_start(xt[:, 1:nt], x_d.ap()[:, 1:nt])
                        nc.sync.dma_start(w9dec[:], w9dec_d.ap())
                        nc.sync.dma_start(wbg0t[:], wbg0t_d.ap())
                        nc.sync.dma_start(wbgfc[:], wbgfc_d.ap())
                        nc.sync.dma_start(wfcd[:], wfcd_d.ap())
                        nc.sync.dma_start(bg[:], bg0_d.ap())
                    else:
                        nc.sync.dma_start(xt[:, :nt], x_d.ap()[:, t : t + nt])
                tr = t - t0
                for layer in range(2):
                    quantum(t, layer, xt, tr)

            # ---- batched fc head over all decode steps (off the loop) ----
            ps_fc = psum.tile([T_DEC, NS, SB], f32, tag="psA", name="ps_fc")
            for td in range(T_DEC):
                for st in range(NS):
                    nc.tensor.matmul(
                        ps_fc[:, st, :], wfcd[:, td, :], h1dec[:, td, st, :],
                        start=td == 0, stop=td == T_DEC - 1,
                    )
            pred = gates.tile([T_DEC, NS, SB], f32, tag="pred")
            nc.scalar.activation(pred[:], ps_fc[:], COPY, bias=bfc)
            nc.sync.dma_start(out_d.ap()[:], pred[:])

    nc.compile()
    return nc


def _get_nc(bfc: float):
    if _CACHE.get("bfc") != bfc:
        _CACHE["nc"] = _build(bfc)
        _CACHE["bfc"] = bfc
    return _CACHE["nc"]


def _reord(w):
    """reorder gate columns from pytorch [i,f,g,o] to kernel [g,i,f,o]."""
    i, f, g, o = (w[..., k * H : (k + 1) * H] for k in range(4))
    return np.concatenate([g, i, f, o], axis=-1)


def kernel(
    inputs,
    W_ih_0, W_hh_0, b_ih_0, b_hh_0,
    W_ih_1, W_hh_1, b_ih_1, b_hh_1,
    W_fc, b_fc,
):
    inputs = np.asarray(inputs, np.float32)
    bfc = float(np.asarray(b_fc).reshape(-1)[0])
    nc = _get_nc(bfc)

    b0 = (b_ih_0 + b_hh_0).astype(np.float32)
    bfc32 = np.float32(bfc)
    w9t0 = _reord(
        np.concatenate([W_ih_0.T.astype(np.float32), b0[None, :]], axis=0)
    ).astype(F16)  # [9, 512]; row 8 is the bias
    w9t0 = np.concatenate([w9t0, np.zeros((KX - DIN - 1, NG), F16)], axis=0)
    # decode variant: bias row also carries b_fc * W_bg (the feedback
    # matmul delivers the raw fc output, without b_fc)
    w9dec = _reord(
        np.concatenate(
            [W_ih_0.T.astype(np.float32),
             (b0 + bfc32 * W_ih_0[:, 0].astype(np.float32))[None, :]], axis=0
        )
    ).astype(F16)
    w9dec = np.concatenate([w9dec, np.zeros((KX - DIN - 1, NG), F16)], axis=0)
    whht0 = _reord(W_hh_0.T.astype(np.float32)).astype(F16)
    wiht1 = _reord(W_ih_1.T.astype(np.float32)).astype(F16)
    whht1 = _reord(W_hh_1.T.astype(np.float32)).astype(F16)
    wbg0t = np.repeat(
        _reord(W_ih_0.T[0:1, :].astype(np.float32)) / H, H, axis=0
    ).astype(F16)  # [128, 512], K=128 ones-style part keeps the PE array hot
    # rank-1 fold of the fc head through the BG column: gate j gets
    # W_ih_0[j,0] * (W_fc . h1)
    wbgfc = _reord(
        np.outer(W_fc.astype(np.float32)[0], W_ih_0[:, 0].astype(np.float32))
    ).astype(F16)  # [128, 512]
    b1v = (b_ih_1 + b_hh_1).astype(np.float32)  # [512] in [i,f,g,o] order
    b1 = np.stack(
        [b1v[2 * H : 3 * H], b1v[0:H], b1v[H : 2 * H], b1v[3 * H :]], axis=1
    ).astype(np.float32)  # [128, 4] in slot order g,i,f,o
    wfcd = np.zeros((H, T_DEC, T_DEC), np.float32)
    for td in range(T_DEC):
        wfcd[:, td, td] = W_fc.astype(np.float32)[0]
    wfcd = wfcd.astype(F16)

    in_maps = []
    for i in range(N_CORES):
        sh = inputs[i * BSH : (i + 1) * BSH]  # [1024, 60, 8]
        x = np.ascontiguousarray(sh.transpose(2, 1, 0))  # [8, 60, 1024]
        x9 = np.concatenate(
            [x, np.ones((1, T, BSH), np.float32),
             np.zeros((KX - DIN - 1, T, BSH), np.float32)], axis=0
        )  # [128, 60, 1024]; rows 9+ are zero (PE activity padding)
        x9[0, T_ENC:, :] = 0.0  # BG channel rides the feedback matmul in decode
        bg0 = sh[:, T_ENC, 0].reshape(1, BSH)
        in_maps.append(
            {
                "x": x9.astype(F16).reshape(KX, T, NS, SB),
                "w9t0": w9t0,
                "w9dec": w9dec,
                "whht0": whht0,
                "wiht1": wiht1,
                "whht1": whht1,
                "wbg0t": wbg0t,
                "wbgfc": wbgfc,
                "wfcd": wfcd,
                "b1": b1,
                "bg0": np.repeat(bg0, H, axis=0).reshape(H, NS, SB).astype(F16),
            }
        )

    res = bass_utils.run_bass_kernel_spmd(
        nc, in_maps, core_ids=list(range(N_CORES))
    )
    outs = []
    for i in range(N_CORES):
        o = res.results[i]["out"].reshape(T_DEC, BSH)  # [12, 1024] fp32
        outs.append(o.T[:, :, None])  # [1024, 12, 1]
    return np.concatenate(outs, axis=0).astype(np.float32)


if __name__ == "__main__":
    _get_nc(0.0)
    print("build + compile OK")


# revision 34
# speedup vs baseline: 1.0024x; 1.0024x over previous
"""Trainium2 Bass kernel for a 2-layer LSTM encoder + autoregressive decoder.

Problem: batch 8192, 48 encode steps, 12 decode steps with BG-channel
feedback, hidden 128, input dim 8, fc head to 1 output.

Strategy: pure data parallelism over 8 NeuronCores (1024 batch rows each).
Hidden units sit on SBUF partitions, batch (1024) on the free dim in two
512-wide matmul streams (ISA moving-dim cap is 512):

  gates[512, B] = W_ih^T.T @ x[K, B] + W_hh^T.T @ h[128, B]   (PSUM accum)

The kernel is ScalarE(ACT)-bound: the 5 LUT activations per layer-step
over [128, 1024] are ~5us of mandatory ACT-engine streaming (1 elem/
cycle/lane, LUT exists only on ScalarE).  Everything serves keeping
ScalarE fed:

- x is zero-padded from 9 to 128 contraction rows (KX): K=9 matmuls run
  the PE array at 7% activity, which parks the HAM clock-gate at half
  rate for the whole loop; uniform K=128 work holds the PE at 2.4 GHz
  (216ns vs 427+ns per N=512 matmul).
- Per-gate 2-bank PSUM tiles, 2 tags x 2 bufs = all 8 banks, rolling
  per-gate across layer-steps.  ScalarE order [g, i, f, o, c] lets the
  DVE c-chain (t1, u, add) drain under sig_f/sig_o.
- Layer 1 emits all always-ready h1-part matmuls before the fresh-h0
  x-parts (the in-order PE queue would head-of-line-block otherwise).
- The c-update/tanh(c)/h tail is stream-split on the recurrence-critical
  paths (layer 0 always; layer 1 in decode, where h1 feeds the BG
  feedback) so the next quantum's matmuls start half an op earlier.
- Decode BG feedback is folded through the BG column as the rank-1
  weight wbgfc = outer(W_fc, W_bg) applied to last-step h1; b_fc * W_bg
  is folded into w9dec's bias row.  Feedback matmuls ride last in each
  gate's accumulation group (their rhs lands latest).
- Layer-0 biases ride x's 9th channel; layer-1 biases use the ACT
  per-partition bias operand.  Zero h/c at t=0 skip their matmuls.
- The fc head runs off-loop over all 12 decode steps with diagonal-
  extended weights [128, 12] accumulating into one [12, 1024] PSUM tile
  -> a single ACT copy (+b_fc) -> DMA out.
- Startup: one DMA queue drains serially, so step-0's critical tensors
  (w9t0, whht0, x step-0 chunk) are issued first and decode-only
  tensors last; first activation fires at ~9us instead of ~15us.

Weight columns are reordered host-side to [g, i, f, o].
Measured: 753us HW exec (baseline 814us), rel err 1.24e-3.  ScalarE is
86% busy on ~640us of LUT streaming; the residual idle is pure recurrence
dependency latency (the ~1.1us h-turnaround per step has no fillable work:
ScalarE is packed tail-to-head up to each gap, so added ops anywhere only
relocate it -- measured twice).
"""

import sys

sys.path.insert(0, "/opt/trn_rl_repo")

import numpy as np

import concourse.bacc as bacc
import concourse.tile as tile
from concourse import mybir
from concourse import bass_utils
from concourse.bass import ts

F16 = np.float16

B_TOTAL = 8192
T = 60
T_ENC = 48
T_DEC = 12
DIN = 8
H = 128
NG = 4 * H
N_CORES = 8
BSH = B_TOTAL // N_CORES  # 1024 batch rows per core
NS = 2  # batch streams
SB = BSH // NS  # 512
XT_STEPS = 8  # timesteps per streamed x tile
KX = 128  # x rows zero-padded 9 -> 128: K=9 matmuls run the PE array at 7%
          # activity, which keeps the HAM clock-gate at half rate; uniform
          # K=128 work holds the PE at 2.4 GHz

# gate slots after host-side column reorder: g, i, f, o
SG, SI, SF, SO = 0, 1, 2, 3

_CACHE: dict = {}


def _build(bfc: float):
    f32 = mybir.dt.float32
    f16 = mybir.dt.float16

    nc = bacc.Bacc("TRN2", debug=False, num_devices=N_CORES)

    x_d = nc.dram_tensor("x", [KX, T, NS, SB], f16, kind="ExternalInput")
    w9t0_d = nc.dram_tensor("w9t0", [KX, NG], f16, kind="ExternalInput")
    w9dec_d = nc.dram_tensor("w9dec", [KX, NG], f16, kind="ExternalInput")
    whht0_d = nc.dram_tensor("whht0", [H, NG], f16, kind="ExternalInput")
    wiht1_d = nc.dram_tensor("wiht1", [H, NG], f16, kind="ExternalInput")
    whht1_d = nc.dram_tensor("whht1", [H, NG], f16, kind="ExternalInput")
    wbg0t_d = nc.dram_tensor("wbg0t", [H, NG], f16, kind="ExternalInput")
    wbgfc_d = nc.dram_tensor("wbgfc", [H, NG], f16, kind="ExternalInput")
    wfcd_d = nc.dram_tensor("wfcd", [H, T_DEC, T_DEC], f16, kind="ExternalInput")
    b1_d = nc.dram_tensor("b1", [H, 4], f32, kind="ExternalInput")
    bg0_d = nc.dram_tensor("bg0", [H, NS, SB], f16, kind="ExternalInput")
    out_d = nc.dram_tensor("out", [T_DEC, NS, SB], f32, kind="ExternalOutput")

    SIG = mybir.ActivationFunctionType.Sigmoid
    TANH = mybir.ActivationFunctionType.Tanh
    COPY = mybir.ActivationFunctionType.Copy

    with tile.TileContext(nc) as tc:
        with (
            tc.tile_pool(name="wpool", bufs=1) as wpool,
            tc.tile_pool(name="xpool", bufs=3) as xpool,
            tc.tile_pool(name="state", bufs=1) as state,
            tc.tile_pool(name="gates", bufs=3) as gates,
            tc.tile_pool(name="psum", bufs=2, space="PSUM") as psum,
        ):
            w9t0 = wpool.tile([KX, NG], f16)
            w9dec = wpool.tile([KX, NG], f16)
            whht0 = wpool.tile([H, NG], f16)
            wiht1 = wpool.tile([H, NG], f16)
            whht1 = wpool.tile([H, NG], f16)
            wbg0t = wpool.tile([H, NG], f16)
            wbgfc = wpool.tile([H, NG], f16)
            wfcd = wpool.tile([H, T_DEC, T_DEC], f16)
            b1 = wpool.tile([H, 4], f32)
            # Startup: a dummy sigmoid first makes the one table set that
            # holds BOTH sigmoid and tanh resident in a single
            # ACT_TABLE_LOAD (tanh-first loads tanh's home set, then the
            # sigmoid set replaces it: two loads).  Junk matmuls on the
            # scratch tile warm the PE clock-gate to 2.4 GHz while the
            # first weight/x DMAs drain.
            scratch = wpool.tile([H, SB], f16)
            nc.vector.memset(scratch[:], 0.25)
            nc.scalar.activation(scratch[:, 1:2], scratch[:, 0:1], SIG)
            ps_warm = psum.tile([H, NS, SB], f32, tag="psA", name="warmup")
            for k in range(4):
                nc.tensor.matmul(
                    ps_warm[:, 0, :], scratch[:, 0:H], scratch[:],
                    start=k == 0, stop=k == 3,
                )
            # DMA order matters: one queue drains these serially, so the
            # step-0 critical path (w9t0, whht0, then the first x chunk,
            # issued below) goes first; decode-only tensors go last.
            nc.sync.dma_start(w9t0[:], w9t0_d.ap())
            nc.sync.dma_start(whht0[:], whht0_d.ap())

            # recurrent state; h double-buffered on step parity
            h = [[None, None], [None, None]]  # h[layer][parity]
            c = [None, None]
            for l in range(2):
                for p in range(2):
                    h[l][p] = state.tile([H, NS, SB], f16, name=f"h_{l}_{p}")
                    nc.vector.memset(h[l][p][:], 0.0)
                c[l] = state.tile([H, NS, SB], f16, name=f"c_{l}")
                nc.vector.memset(c[l][:], 0.0)
            # decode h1 states kept for the batched fc head
            h1dec = state.tile([H, T_DEC, NS, SB], f16, name="h1dec")
            bg = state.tile([H, NS, SB], f16, name="bg")

            def h1_read(t):
                """h1 produced at step t-1 (as read by step t)."""
                if t <= T_ENC:
                    return h[1][1 - (t % 2)]
                return h1dec[:, t - 1 - T_ENC]

            def quantum(t, layer, xt, tr):
                p = t % 2
                dec = t >= T_ENC

                # Per-gate 2-bank PSUM tiles [H, NS, SB]; 2 tags x 2 bufs
                # = all 8 banks, rolling per-gate across quanta.
                gps = {}
                for slot, tag in ((SG, "psA"), (SI, "psB"), (SF, "psA"), (SO, "psB")):
                    gps[slot] = psum.tile(
                        [H, NS, SB], f32, tag=tag, name=f"ps_{t}_{layer}_{slot}"
                    )

                # ---- matmuls (all K=128, N=512) ----
                if layer == 0:
                    w_x = w9dec if t > T_ENC else w9t0
                    h_prev = h[0][1 - p]
                    # gate-major, x parts then h parts per gate: everything
                    # is ready, so tanh(g) can fire after the first 4-6 MMs
                    for slot in (SG, SI, SF, SO):
                        for st in range(NS):
                            nc.tensor.matmul(
                                gps[slot][:, st, :], w_x[:, ts(slot, H)],
                                xt[:, tr, st, :], start=True, stop=False,
                            )
                        for st in range(NS):
                            nc.tensor.matmul(
                                gps[slot][:, st, :], whht0[:, ts(slot, H)],
                                h_prev[:, st, :], start=False,
                                stop=not (dec or t == T_ENC),
                            )
                        if t == T_ENC:
                            for st in range(NS):
                                nc.tensor.matmul(
                                    gps[slot][:, st, :], wbg0t[:, ts(slot, H)],
                                    bg[:, st, :], start=False, stop=True,
                                )
                        elif dec:
                            # feedback last: its rhs (h1[t-1]) lands latest
                            for st in range(NS):
                                nc.tensor.matmul(
                                    gps[slot][:, st, :], wbgfc[:, ts(slot, H)],
                                    h1_read(t)[:, st, :], start=False, stop=True,
                                )
                else:
                    # layer 1: all always-ready h1-parts first, then the
                    # fresh-h0 x-parts (the in-order PE queue would
                    # head-of-line-block on an early fresh-h0 matmul)
                    h_prev = h1_read(t)
                    h0_t = h[0][p]
                    if t > 0:  # h1 is zero at t=0
                        for slot in (SG, SI, SF, SO):
                            for st in range(NS):
                                nc.tensor.matmul(
                                    gps[slot][:, st, :], whht1[:, ts(slot, H)],
                                    h_prev[:, st, :], start=True, stop=False,
                                )
                    for slot in (SG, SI, SF, SO):
                        for st in range(NS):
                            nc.tensor.matmul(
                                gps[slot][:, st, :], wiht1[:, ts(slot, H)],
                                h0_t[:, st, :], start=t == 0, stop=True,
                            )

                # ---- activations + cell update (FD=1024 ops) ----
                if layer == 1 and dec:
                    h_new = h1dec[:, t - T_ENC]
                else:
                    h_new = h[layer][p]
                c_own = c[layer]

                g_sb = gates.tile([H, NS, SB], f16, tag="g")
                i_sb = gates.tile([H, NS, SB], f16, tag="i")
                f_sb = gates.tile([H, NS, SB], f16, tag="f")
                o_sb = gates.tile([H, NS, SB], f16, tag="o")
                th = gates.tile([H, NS, SB], f16, tag="th")
                t1 = gates.tile([H, NS, SB], f16, tag="t1")
                u = gates.tile([H, NS, SB], f16, tag="u")

                bias = {}
                if layer == 1:
                    bias = {s: {"bias": b1[:, s : s + 1]} for s in range(4)}
                # ScalarE order [g, i, f, o, c]: the DVE chain (t1, u, add)
                # runs under sig_f/sig_o, so tanh(c) is ready when reached.
                # The h-multiply splits per stream so stream A's x-matmuls
                # of the next quantum start half an h-op earlier.  (Splitting
                # tanh_g per stream was tried twice and lost: +2 ops/step of
                # overhead beats the sub-op latency gain.)
                nc.scalar.activation(g_sb[:], gps[SG][:], TANH, **bias.get(SG, {}))
                nc.scalar.activation(i_sb[:], gps[SI][:], SIG, **bias.get(SI, {}))
                nc.vector.tensor_mul(t1[:], i_sb[:], g_sb[:])
                nc.scalar.activation(f_sb[:], gps[SF][:], SIG, **bias.get(SF, {}))
                if layer == 0:
                    # layer 0 feeds the critical h0 turnaround: run the
                    # c-update, tanh(c) and h per stream so stream A's
                    # next-quantum matmuls start ~0.5us earlier
                    for st in range(NS):
                        nc.vector.tensor_mul(
                            u[:, st, :], f_sb[:, st, :], c_own[:, st, :]
                        )
                        nc.vector.tensor_add(
                            c_own[:, st, :], u[:, st, :], t1[:, st, :]
                        )
                    nc.scalar.activation(o_sb[:], gps[SO][:], SIG)
                    for st in range(NS):
                        nc.scalar.activation(th[:, st, :], c_own[:, st, :], TANH)
                        nc.vector.tensor_mul(
                            h_new[:, st, :], o_sb[:, st, :], th[:, st, :]
                        )
                elif dec:
                    # decode: h1 feeds next step's BG feedback, so stream-
                    # split the tail for latency like layer 0
                    for st in range(NS):
                        nc.vector.tensor_mul(
                            u[:, st, :], f_sb[:, st, :], c_own[:, st, :]
                        )
                        nc.vector.tensor_add(
                            c_own[:, st, :], u[:, st, :], t1[:, st, :]
                        )
                    nc.scalar.activation(
                        o_sb[:], gps[SO][:], SIG, **bias.get(SO, {})
                    )
                    for st in range(NS):
                        nc.scalar.activation(th[:, st, :], c_own[:, st, :], TANH)
                        nc.vector.tensor_mul(
                            h_new[:, st, :], o_sb[:, st, :], th[:, st, :]
                        )
                else:
                    # encode layer 1: h1 is only needed a full step later,
                    # so use the cheaper merged FD=1024 ops
                    nc.vector.tensor_mul(u[:], f_sb[:], c_own[:])
                    nc.vector.tensor_add(c_own[:], u[:], t1[:])
                    nc.scalar.activation(
                        o_sb[:], gps[SO][:], SIG, **bias.get(SO, {})
                    )
                    nc.scalar.activation(th[:], c_own[:], TANH)
                    nc.vector.tensor_mul(h_new[:], o_sb[:], th[:])

            # x tiles: 8-step tiles for encode, one 12-step tile for decode.
            # The first tile's DMA is split so step 0 starts after ~1/8 of
            # the transfer instead of the whole 2MB.
            xt = None
            t0 = 0
            x_tile_starts = list(range(0, T_ENC, XT_STEPS)) + [T_ENC]
            for t in range(T):
                if t in x_tile_starts:
                    t0 = t
                    nt = T_DEC if t == T_ENC else XT_STEPS
                    xt = xpool.tile([KX, T_DEC, NS, SB], f16)
                    if t == 0:
                        nc.sync.dma_start(xt[:, :1], x_d.ap()[:, 0:1])
                        nc.sync.dma_start(wiht1[:], wiht1_d.ap())
                        nc.sync.dma_start(whht1[:], whht1_d.ap())
                        nc.sync.dma_start(b1[:], b1_d.ap())
                        nc.sync.dma_start(xt[:, 1:nt], x_d.ap()[:, 1:nt])
                        nc.sync.dma_start(w9dec[:], w9dec_d.ap())
                        nc.sync.dma_start(wbg0t[:], wbg0t_d.ap())
                        nc.sync.dma_start(wbgfc[:], wbgfc_d.ap())
                        nc.sync.dma_start(wfcd[:], wfcd_d.ap())
                        nc.sync.dma_start(bg[:], bg0_d.ap())
                    else:
                        nc.sync.dma_start(xt[:, :nt], x_d.ap()[:, t : t + nt])
                tr = t - t0
                for layer in range(2):
                    quantum(t, layer, xt, tr)

            # ---- batched fc head over all decode steps (off the loop) ----
            ps_fc = psum.tile([T_DEC, NS, SB], f32, tag="psA", name="ps_fc")
            for td in range(T_DEC):
                for st in range(NS):
                    nc.tensor.matmul(
                        ps_fc[:, st, :], wfcd[:, td, :], h1dec[:, td, st, :],
                        start=td == 0, stop=td == T_DEC - 1,
                    )
            pred = gates.tile([T_DEC, NS, SB], f32, tag="pred")
            nc.scalar.activation(pred[:], ps_fc[:], COPY, bias=bfc)
            nc.sync.dma_start(out_d.ap()[:], pred[:])

    nc.compile()
    return nc


def _get_nc(bfc: float):
    if _CACHE.get("bfc") != bfc:
        _CACHE["nc"] = _build(bfc)
        _CACHE["bfc"] = bfc
    return _CACHE["nc"]


def _reord(w):
    """reorder gate columns from pytorch [i,f,g,o] to kernel [g,i,f,o]."""
    i, f, g, o = (w[..., k * H : (k + 1) * H] for k in range(4))
    return np.concatenate([g, i, f, o], axis=-1)


def kernel(
    inputs,
    W_ih_0, W_hh_0, b_ih_0, b_hh_0,
    W_ih_1, W_hh_1, b_ih_1, b_hh_1,
    W_fc, b_fc,
):
    inputs = np.asarray(inputs, np.float32)
    bfc = float(np.asarray(b_fc).reshape(-1)[0])
    nc = _get_nc(bfc)

    b0 = (b_ih_0 + b_hh_0).astype(np.float32)
    bfc32 = np.float32(bfc)
    w9t0 = _reord(
        np.concatenate([W_ih_0.T.astype(np.float32), b0[None, :]], axis=0)
    ).astype(F16)  # [9, 512]; row 8 is the bias
    w9t0 = np.concatenate([w9t0, np.zeros((KX - DIN - 1, NG), F16)], axis=0)
    # decode variant: bias row also carries b_fc * W_bg (the feedback
    # matmul delivers the raw fc output, without b_fc)
    w9dec = _reord(
        np.concatenate(
            [W_ih_0.T.astype(np.float32),
             (b0 + bfc32 * W_ih_0[:, 0].astype(np.float32))[None, :]], axis=0
        )
    ).astype(F16)
    w9dec = np.concatenate([w9dec, np.zeros((KX - DIN - 1, NG), F16)], axis=0)
    whht0 = _reord(W_hh_0.T.astype(np.float32)).astype(F16)
    wiht1 = _reord(W_ih_1.T.astype(np.float32)).astype(F16)
    whht1 = _reord(W_hh_1.T.astype(np.float32)).astype(F16)
    wbg0t = np.repeat(
        _reord(W_ih_0.T[0:1, :].astype(np.float32)) / H, H, axis=0
    ).astype(F16)  # [128, 512], K=128 ones-style part keeps the PE array hot
    # rank-1 fold of the fc head through the BG column: gate j gets
    # W_ih_0[j,0] * (W_fc . h1)
    wbgfc = _reord(
        np.outer(W_fc.astype(np.float32)[0], W_ih_0[:, 0].astype(np.float32))
    ).astype(F16)  # [128, 512]
    b1v = (b_ih_1 + b_hh_1).astype(np.float32)  # [512] in [i,f,g,o] order
    b1 = np.stack(
        [b1v[2 * H : 3 * H], b1v[0:H], b1v[H : 2 * H], b1v[3 * H :]], axis=1
    ).astype(np.float32)  # [128, 4] in slot order g,i,f,o
    wfcd = np.zeros((H, T_DEC, T_DEC), np.float32)
    for td in range(T_DEC):
        wfcd[:, td, td] = W_fc.astype(np.float32)[0]
    wfcd = wfcd.astype(F16)

    in_maps = []
    for i in range(N_CORES):
        sh = inputs[i * BSH : (i + 1) * BSH]  # [1024, 60, 8]
        x = np.ascontiguousarray(sh.transpose(2, 1, 0))  # [8, 60, 1024]
        x9 = np.concatenate(
            [x, np.ones((1, T, BSH), np.float32),
             np.zeros((KX - DIN - 1, T, BSH), np.float32)], axis=0
        )  # [128, 60, 1024]; rows 9+ are zero (PE activity padding)
        x9[0, T_ENC:, :] = 0.0  # BG channel rides the feedback matmul in decode
        bg0 = sh[:, T_ENC, 0].reshape(1, BSH)
        in_maps.append(
            {
                "x": x9.astype(F16).reshape(KX, T, NS, SB),
                "w9t0": w9t0,
                "w9dec": w9dec,
                "whht0": whht0,
                "wiht1": wiht1,
                "whht1": whht1,
                "wbg0t": wbg0t,
                "wbgfc": wbgfc,
                "wfcd": wfcd,
                "b1": b1,
                "bg0": np.repeat(bg0, H, axis=0).reshape(H, NS, SB).astype(F16),
            }
        )

    res = bass_utils.run_bass_kernel_spmd(
        nc, in_maps, core_ids=list(range(N_CORES))
    )
    outs = []
    for i in range(N_CORES):
        o = res.results[i]["out"].reshape(T_DEC, BSH)  # [12, 1024] fp32
        outs.append(o.T[:, :, None])  # [1024, 12, 1]
    return np.concatenate(outs, axis=0).astype(np.float32)


if __name__ == "__main__":
    _get_nc(0.0)
    print("build + compile OK")
